# revision 1
# baseline (speedup 1.0000x reference)
"""Trainium2 Bass kernel for nn_NeuralODE_38053410242883.

Neural ODE: x_{k+1} = x_k + eps*f(x_k, u_k) scanned over T=100000 steps
(f = MLP 3->32->32->2, softplus), then readout y = g(x) (MLP 2->16->1).

Strategy: parallel-in-time Picard iteration. The fixed point of
    x_k = x0 + sum_{j<k} eps*f(x_j, u_j)
is the exact Euler trajectory; each sweep is a fully parallel batched MLP
over all timesteps plus a prefix sum, and contracts by ~1e-2 per sweep
(3 sweeps land at ~3e-5 rel err, dominated by the exp/ln ACT tables).
Time is sharded over 8 NeuronCores (12500 steps each); the only
communication is an AllGather of per-core increment totals ([8] floats)
per sweep, and everything AG-independent (next sweep's layer-1 matmuls,
its bias-free Exp pass, the f32r rounding copy) is arranged to execute
under the AllGather. Softplus is computed as Ln(Exp(z)+1) (both functions
live in one ACT table set; no native softplus table exists), with the
layer-1 bias folded in multiplicatively via Ln's per-partition scale
operand (softplus(z+b) = Ln(e^z*e^b + 1)) so the Exp pass needs no
AllGather result. Big matmuls use the f32r PE fast path (1 cycle/row vs
4 for fp32); every f32r operand is produced rounded (DMA into f32r
tiles, ACT/DVE writes with f32r output dtype), which measured at
fp32-class accuracy, while the readout's final matmul stays fp32
(directly visible in y).

Per-core layout: 12500 steps padded to 12800 = 4 blocks x 3200 cols.
Activations keep features on partitions, time on the free dim; the 4
blocks are stacked along partitions via block-diagonal weights
(z [12,3200] -> h [128,3200] -> h [128,3200] -> d [8,3200]).
The prefix sum runs as chained vector-engine tensor_tensor_scan ops
reading the L3 PSUM output directly, with eps*b3 folded in via the scan's
second operand (masked so padded steps contribute zero). All constant
offsets (x0 + cross-core prefix + intra-core block prefix) are folded
into layer-1's activation bias through tiny matmuls, so z's x-rows hold
only the local in-block prefix.
"""

import sys

import numpy as np

if "/opt/trn_rl_repo" not in sys.path:
    sys.path.insert(0, "/opt/trn_rl_repo")

import concourse.bacc as bacc
import concourse.tile as tile
from concourse import mybir
from concourse.bass_utils import run_bass_kernel_spmd

F32 = mybir.dt.float32
AF = mybir.ActivationFunctionType
ALU = mybir.AluOpType
F32R = mybir.dt.float32r


# Both Exp and Ln live in the natural_log_exp_and_others ACT table set, but
# the table-load inserter picks the first set containing each function,
# which alternates exp_and_others / natural_log and pays the ~2.7us table
# switch on every activation. Strip Exp/Ln from every other set (preserving
# set order, which act_func_set_id indexes) so both resolve to the shared set.
_GAT_ORIG = bacc.get_activation_tables


def _gat_patched(arch):
    tables = _GAT_ORIG(arch)
    for name, funcs in tables.items():
        if name != "natural_log_exp_and_others":
            funcs.discard(AF.Exp)
            funcs.discard(AF.Ln)
    return tables


bacc.get_activation_tables = _gat_patched

NCORES = 8
T = 100000
S = 12500          # valid steps per core
B = 3200           # steps per block (4 blocks per core)
Q = 4              # blocks per core
PADC = 2900        # valid cols in block 3 (B*3 + PADC = S)
NIT = 2            # Picard sweeps (sweep-2 residual ~4e-4, far below tolerance)
WID = 32
GW = 16

# matmul column slices over the 3200-wide free dim (<=512 each)
MM_SLICES = [(i * 512, 512) for i in range(6)] + [(3072, 128)]
# ACT groups (psum tiles for L1/L2 pre-activations), <=1024 wide
ACT_SLICES = [(0, 1024), (1024, 1024), (2048, 1024), (3072, 128)]

_CACHE = {}


def _build_program():
    nc = bacc.Bacc("TRN2", target_bir_lowering=False, debug=False,
                   num_devices=NCORES)

    dram = {}
    def din(name, shape, dt=F32):
        dram[name] = nc.dram_tensor(name, list(shape), dt,
                                    kind="ExternalInput").ap()
    din("constsR", (128, 328), F32R)   # packed f32r weights, one DMA
    din("constsF", (128, 153))         # packed fp32 constants, one DMA
    din("c3m", (8, B))
    din("u4", (4, B), F32R)
    out = nc.dram_tensor("out", [S], F32, kind="ExternalOutput").ap()

    with tile.TileContext(nc) as tc:
        with (
            tc.tile_pool(name="const", bufs=1) as cpool,
            tc.tile_pool(name="h", bufs=4) as hpool,
            tc.tile_pool(name="bias", bufs=2) as bpool,
            tc.tile_pool(name="hpre", bufs=2, space="PSUM") as hpre,
            tc.tile_pool(name="psmall", bufs=2, space="PSUM") as psmall,
            tc.tile_pool(name="dram", bufs=2, space="DRAM") as dpool,
        ):
            # ---- load constants (two packed DMAs: f32r / fp32) ----
            CR = cpool.tile([128, 328], F32R, tag="constsR")
            nc.sync.dma_start(out=CR[:], in_=dram["constsR"])
            W2bd = CR[0:128, 0:128]
            W1bd = CR[0:12, 128:256]
            W3bde = CR[0:128, 256:264]
            Wg1bd = CR[0:8, 264:328]
            CF = cpool.tile([128, 153], F32, tag="constsF")
            nc.sync.dma_start(out=CF[:], in_=dram["constsF"])
            M1T = CF[0:8, 0:128]
            Tq = CF[0:8, 128:136]
            maskC = CF[0:64, 136:144]
            extras = CF[0:8, 144:145]
            b1bd = CF[0:128, 145:146]
            b2bd = CF[0:128, 146:147]
            bg1bd = CF[0:64, 147:148]
            Wg2bd = CF[0:64, 148:152]
            bg2b = CF[0:4, 152:153]
            c3m_t = cpool.tile([8, B], F32, tag="c3m")
            nc.sync.dma_start(out=c3m_t[:], in_=dram["c3m"])
            c3m = c3m_t

            # z: rows 0-7 = x local prefix (2q+f), rows 8-11 = u (block q)
            z_r = cpool.tile([12, B], F32R, tag="z_r")
            nc.vector.memset(z_r[0:8, :].bitcast(F32), 0.0)
            nc.sync.dma_start(out=z_r[8:12, :], in_=dram["u4"])
            # z cols 1: are scan-written before any read; only col 0 (the
            # x_local(q,0)=0 boundary) needs initialization
            z = cpool.tile([8, B], F32, tag="z")
            nc.vector.memset(z[:, 0:1], 0.0)

            def l1_matmuls(first=False):
                """Layer-1 matmuls into fresh psum tiles (AG-independent)."""
                tiles = {}
                for (c0, w) in ACT_SLICES:
                    p1 = hpre.tile([128, 1024], F32, tag="hpre")
                    for s0 in range(0, w, 512):
                        sw = min(512, w - s0)
                        nc.tensor.matmul(p1[:, s0:s0 + sw], W1bd[:],
                                         z_r[:, c0 + s0:c0 + s0 + sw],
                                         start=True, stop=True)
                    tiles[c0] = p1
                return tiles

            def ebias_chain(rowtot, agout):
                """e^{layer-1 bias} from the offset fold-in.

                basex = Tq.T@rowtot + maskC.T@agout + extras (= x0 + global
                prefix + block prefix); ebias = Exp(M1T.T@basex + b1bd).
                With rowtot/agout None (first sweep): basex = extras.
                """
                bx = psmall.tile([8, 1], F32, tag="ptiny")
                basex_sb = bpool.tile([8, 1], F32, tag="basex")
                if rowtot is not None:
                    nc.tensor.matmul(bx[:], Tq[:], rowtot[:], start=True,
                                     stop=False)
                    nc.tensor.matmul(bx[:], maskC[:], agout[:], start=False,
                                     stop=True)
                    nc.vector.tensor_scalar(basex_sb[:], bx[:],
                                            extras[:, 0:1], None, ALU.add)
                else:
                    nc.vector.tensor_copy(basex_sb[:], extras[:])
                bp = psmall.tile([128, 1], F32, tag="ptiny")
                nc.tensor.matmul(bp[:], M1T[:], basex_sb[:], start=True,
                                 stop=True)
                ebias = bpool.tile([128, 1], F32, tag="ebias")
                nc.scalar.activation(ebias[:], bp[:], AF.Exp,
                                     bias=b1bd[:, 0:1])
                return basex_sb, ebias

            p1t = l1_matmuls(first=True)
            prev = (None, None)
            basex_sb = None
            for it in range(NIT):
                last = it == NIT - 1
                # Exp of layer 1, bias-free (overlaps the previous AllGather)
                e1 = {}
                for (c0, w) in ACT_SLICES:
                    t1e = hpool.tile([128, 1024], F32, tag="h1e")
                    nc.scalar.activation(t1e[:, 0:w], p1t[c0][:, 0:w], AF.Exp)
                    e1[c0] = t1e
                # offset fold-in (needs the AllGather result)
                basex_sb, ebias = ebias_chain(*prev)
                # rest of layer 1: h1 = Ln(e^{W1z} * e^{bias} + 1)
                h1 = {}
                for (c0, w) in ACT_SLICES:
                    t1 = hpool.tile([128, 1024], F32R, tag="h1")
                    nc.scalar.activation(t1[:, 0:w], e1[c0][:, 0:w], AF.Ln,
                                         bias=1.0, scale=ebias[:, 0:1])
                    h1[c0] = t1
                # layer 2
                h2 = {}
                for (c0, w) in ACT_SLICES:
                    p2 = hpre.tile([128, 1024], F32, tag="hpre")
                    for s0 in range(0, w, 512):
                        sw = min(512, w - s0)
                        nc.tensor.matmul(p2[:, s0:s0 + sw], W2bd[:],
                                         h1[c0][:, s0:s0 + sw],
                                         start=True, stop=True)
                    t2e = hpool.tile([128, 1024], F32, tag="h2e")
                    nc.scalar.activation(t2e[:, 0:w], p2[:, 0:w], AF.Exp,
                                         bias=b2bd[:, 0:1])
                    t2 = hpool.tile([128, 1024], F32R, tag="h2")
                    nc.scalar.activation(t2[:, 0:w], t2e[:, 0:w], AF.Ln,
                                         bias=1.0)
                    # zero block-3 rows at padded steps so d~ = 0 there
                    if c0 <= PADC < c0 + w:
                        nc.vector.memset(t2[96:128, PADC - c0:w].bitcast(F32),
                                         0.0)
                    elif c0 > PADC:
                        nc.vector.memset(t2[96:128, 0:w].bitcast(F32), 0.0)
                    h2[c0] = t2
                # layer 3 + chained prefix scan into z x-rows
                dlast = None
                for gi, (c0, w) in enumerate(MM_SLICES):
                    pd = psmall.tile([8, 512], F32, tag="pd")
                    src = h2[(c0 // 1024) * 1024]
                    so = c0 - (c0 // 1024) * 1024
                    nc.tensor.matmul(pd[:, 0:w], W3bde[:],
                                     src[:, so:so + w], start=True, stop=True)
                    sw = w if gi < len(MM_SLICES) - 1 else w - 1
                    init = 0.0 if gi == 0 else z[0:8, c0:c0 + 1]
                    nc.vector.tensor_tensor_scan(
                        z[0:8, c0 + 1:c0 + sw + 1], pd[:, 0:sw],
                        c3m[:, c0:c0 + sw], init, ALU.add, ALU.add)
                    if gi == len(MM_SLICES) - 1:
                        dlast = pd
                # next sweep's layer-1 matmuls (overlap the AllGather below)
                if not last:
                    p1t = l1_matmuls()
                # rowtot[8,1] = z[:,B-1] + c3m[:,B-1] + d~[:,last]
                rowtot = bpool.tile([8, 1], F32, tag="rowtot")
                nc.vector.scalar_tensor_tensor(
                    rowtot[:], z[0:8, B - 1:B], c3m[:, B - 1:B],
                    dlast[:, 127:128], ALU.add, ALU.add)
                # AllGather core totals
                cc_in = dpool.tile([8, 1], F32, tag="cc_in")
                cc_out = dpool.tile([64, 1], F32, tag="cc_out")
                nc.sync.dma_start(out=cc_in[:], in_=rowtot[:])
                nc.gpsimd.collective_compute(
                    "AllGather", ALU.bypass,
                    replica_groups=[list(range(NCORES))],
                    ins=[cc_in.opt()], outs=[cc_out.opt()])
                agout = bpool.tile([64, 1], F32, tag="agout")
                nc.sync.dma_start(out=agout[:], in_=cc_out[:])
                prev = (rowtot, agout)
                # round the new local prefix for the next sweep's f32r
                # matmuls; runs on DVE during the AllGather
                nc.vector.tensor_copy(z_r[0:8, :], z[:, :])

            # ---- readout: y = Wg2 @ sp(Wg1 @ x + bg1) + bg2 ----
            # Wg1 matmuls + Exp overlap the final AllGather (same trick).
            pgt = {}
            for (c0, w) in ACT_SLICES:
                pg = hpre.tile([64, 1024], F32, tag="hpre")
                for s0 in range(0, w, 512):
                    sw = min(512, w - s0)
                    nc.tensor.matmul(pg[:, s0:s0 + sw], Wg1bd[:],
                                     z_r[0:8, c0 + s0:c0 + s0 + sw],
                                     start=True, stop=True)
                hge = hpool.tile([64, 1024], F32, tag="hge")
                nc.scalar.activation(hge[:, 0:w], pg[:, 0:w], AF.Exp)
                pgt[c0] = hge
            # basex from the final AllGather -> e^{readout bias}
            bxf = psmall.tile([8, 1], F32, tag="ptiny")
            nc.tensor.matmul(bxf[:], Tq[:], prev[0][:], start=True, stop=False)
            nc.tensor.matmul(bxf[:], maskC[:], prev[1][:], start=False,
                             stop=True)
            basex_sb = bpool.tile([8, 1], F32, tag="basex")
            nc.vector.tensor_scalar(basex_sb[:], bxf[:], extras[:, 0:1], None,
                                    ALU.add)
            bgp = psmall.tile([64, 1], F32, tag="ptiny")
            nc.tensor.matmul(bgp[:], Wg1bd[:].bitcast(F32), basex_sb[:],
                             start=True, stop=True)
            ebg = bpool.tile([64, 1], F32, tag="ebg")
            nc.scalar.activation(ebg[:], bgp[:], AF.Exp, bias=bg1bd[:, 0:1])
            y_sb = cpool.tile([4, B], F32, tag="y_sb")
            for (c0, w) in ACT_SLICES:
                hg = hpool.tile([64, 1024], F32, tag="hg")
                nc.scalar.activation(hg[:, 0:w], pgt[c0][:, 0:w], AF.Ln,
                                     bias=1.0, scale=ebg[:, 0:1])
                for s0 in range(0, w, 512):
                    sw = min(512, w - s0)
                    py = psmall.tile([4, 512], F32, tag="ptiny")
                    nc.tensor.matmul(py[:, 0:sw], Wg2bd[:], hg[:, s0:s0 + sw],
                                     start=True, stop=True)
                    nc.vector.tensor_scalar(y_sb[0:4, c0 + s0:c0 + s0 + sw],
                                            py[:, 0:sw], bg2b[:, 0:1], None,
                                            ALU.add)
            nc.sync.dma_start(out=out[0:3 * B].rearrange("(p f) -> p f", f=B),
                              in_=y_sb[0:3, :])
            nc.sync.dma_start(out=out[3 * B:3 * B + PADC],
                              in_=y_sb[3:4, 0:PADC])

    nc.compile()
    return nc


def _prep_in_maps(ts, us, x0, W1, b1, W2, b2, W3, b3, Wg1, bg1, Wg2, bg2):
    f32 = np.float32
    eps = (f32(ts[1]) - f32(ts[0])) * f32(0.001)
    qi = np.arange(Q)

    W1bd = np.zeros((12, 128), f32)
    b1bd = np.zeros(128, f32)
    W2bd = np.zeros((128, 128), f32)
    b2bd = np.zeros(128, f32)
    W3bde = np.zeros((128, 8), f32)
    M1T = np.zeros((8, 128), f32)
    for q in range(Q):
        for f in range(2):
            W1bd[2 * q + f, 32 * q:32 * q + 32] = W1[:, f]
            M1T[2 * q + f, 32 * q:32 * q + 32] = W1[:, f]
        W1bd[8 + q, 32 * q:32 * q + 32] = W1[:, 2]
        b1bd[32 * q:32 * q + 32] = b1
        b2bd[32 * q:32 * q + 32] = b2
        W2bd[32 * q:32 * q + 32, 32 * q:32 * q + 32] = W2.T
        W3bde[32 * q:32 * q + 32, 2 * q:2 * q + 2] = (eps * W3).T

    c3m = np.zeros((8, B), f32)
    for f in range(2):
        c3m[2 * qi + f, :] = eps * b3[f]
    c3m[6:8, PADC:] = 0.0

    Tq = np.zeros((8, 8), f32)
    for qp in range(Q):
        for q in range(Q):
            if qp < q:
                for f in range(2):
                    Tq[2 * qp + f, 2 * q + f] = 1.0

    extras = np.zeros(8, f32)
    for f in range(2):
        extras[2 * qi + f] = x0[f]

    Wg1bd = np.zeros((8, 64), f32)
    bg1bd = np.zeros(64, f32)
    Wg2bd = np.zeros((64, 4), f32)
    for q in range(Q):
        for f in range(2):
            Wg1bd[2 * q + f, 16 * q:16 * q + 16] = Wg1[:, f]
        bg1bd[16 * q:16 * q + 16] = bg1
        Wg2bd[16 * q:16 * q + 16, q] = Wg2[0, :]
    bg2b = np.full(4, bg2[0], f32)

    cr = np.zeros((128, 328), f32)
    cr[0:128, 0:128] = W2bd
    cr[0:12, 128:256] = W1bd
    cr[0:128, 256:264] = W3bde
    cr[0:8, 264:328] = Wg1bd
    cf = np.zeros((128, 153), f32)
    cf[0:8, 0:128] = M1T
    cf[0:8, 128:136] = Tq
    cf[0:8, 144] = extras
    cf[0:128, 145] = b1bd
    cf[0:128, 146] = b2bd
    cf[0:64, 147] = bg1bd
    cf[0:64, 148:152] = Wg2bd
    cf[0:4, 152] = bg2b

    in_maps = []
    for c in range(NCORES):
        maskC = np.zeros((64, 8), f32)
        for r in range(c):
            for qp in range(Q):
                for q in range(Q):
                    for f in range(2):
                        maskC[8 * r + 2 * qp + f, 2 * q + f] = 1.0
        u4 = np.zeros((Q, B), f32)
        u4.reshape(-1)[:S] = us[c * S:(c + 1) * S, 0].astype(f32)
        cfc = cf.copy()
        cfc[0:64, 136:144] = maskC
        in_maps.append(dict(constsR=cr, constsF=cfc, c3m=c3m, u4=u4))
    return in_maps


def kernel(ts, us, x0, W1, b1, W2, b2, W3, b3, Wg1, bg1, Wg2, bg2,
           _collect_perf=None):
    ts = np.asarray(ts, np.float32)
    us = np.asarray(us, np.float32)
    assert ts.shape == (T,) and us.shape == (T, 1) and np.asarray(x0).shape == (2,)

    if "nc" not in _CACHE:
        _CACHE["nc"] = _build_program()
    nc = _CACHE["nc"]

    in_maps = _prep_in_maps(ts, us, np.asarray(x0, np.float32),
                            np.asarray(W1), np.asarray(b1), np.asarray(W2),
                            np.asarray(b2), np.asarray(W3), np.asarray(b3),
                            np.asarray(Wg1), np.asarray(bg1),
                            np.asarray(Wg2), np.asarray(bg2))

    kwargs = dict(_collect_perf) if _collect_perf else {}
    res = run_bass_kernel_spmd(nc, in_maps, core_ids=list(range(NCORES)),
                               **kwargs)
    if _collect_perf is not None:
        _CACHE["last_results"] = res

    y = np.concatenate([res.results[c]["out"] for c in range(NCORES)])
    return y.reshape(T, 1).astype(np.float32)

